# revision 35
# baseline (speedup 1.0000x reference)
"""CLAMSelector kernel for 8 TRN2 NeuronCores (Bass/Tile, SPMD).

Problem: B=4, N=16384, D=512, H=256, C=2; top-k (k=11468) selection over
combined attention + per-class instance-clustering loss.

Sharding: N split across 8 cores (2048 patches each). Per core:
  - fp32 GEMM  h = relu(X@Wa+ba), logits = h@Wbr^T+bbr for its shard
  - AllGather logits (64KB) -> global softmax (poly-exp on DVE, ~2ulp)
  - combined = mean over classes; branchless 28-iter binary search for the
    exact k-th threshold (all 4 batch rows in parallel)
  - local mask + prefix-scan compaction; masked indirect-DMA scatter writes
    only the selected feature rows (OOB positions skipped by bounds_check)
  - instance loss: global top8/bot8 per (b,c) via hierarchical max8/max_index,
    cross-shard row gather via bounds-checked indirect DMA + AllReduce,
    tiny fp32 MLP -> scalar loss
Host assembles full outputs from per-core compacted shards.
"""
import sys
import os

sys.path.insert(0, "/opt/trn_rl_repo")

import json
import numpy as np

import concourse.bass as bass
import concourse.mybir as mybir
from concourse.tile import TileContext
from concourse.bass_utils import run_bass_kernel_spmd

F32 = mybir.dt.float32
BF16 = mybir.dt.bfloat16
I32 = mybir.dt.int32
U32 = mybir.dt.uint32
OP = mybir.AluOpType
AX = mybir.AxisListType
ACTF = mybir.ActivationFunctionType

B, N, D, H, C = 4, 16384, 512, 256, 2
NCORES = 8
NS = N // NCORES          # 2048 patches per core
K = 11468                 # top-k (int(N*0.7))
KK = 8                    # instances per side
SEARCH_ITERS = 20
BIG = 1.0e7

# ---- poly exp constants (exp via 2^k * P(r), |r| <= ln2/2, ~2ulp) ----
LN2_HI = 0.693359375
LN2_LO = -2.12194440e-4
INV_LN2 = 1.4426950408889634
MAGIC = 12582912.0  # 1.5 * 2**23


def _emit_exp(nc, scratch, out, in_, npart):
    """out = exp(in_) elementwise on DVE, fp32 ~2ulp. Deterministic op
    sequence (identical per element regardless of tile shape).
    scratch: dict of 5 preallocated [128, 1024] tiles."""
    t = scratch["t"][:npart, :]
    kf = scratch["kf"][:npart, :]
    r = scratch["r"][:npart, :]
    rr = scratch["rr"][:npart, :]
    ki = scratch["ki"][:npart, :]
    v = nc.vector
    v.tensor_scalar(out=t[:], in0=in_, scalar1=INV_LN2, scalar2=MAGIC,
                    op0=OP.mult, op1=OP.add)
    v.tensor_scalar(out=kf[:], in0=t[:], scalar1=MAGIC, scalar2=None, op0=OP.subtract)
    v.scalar_tensor_tensor(out=r[:], in0=kf[:], scalar=-LN2_HI, in1=in_,
                           op0=OP.mult, op1=OP.add)
    v.scalar_tensor_tensor(out=rr[:], in0=kf[:], scalar=-LN2_LO, in1=r[:],
                           op0=OP.mult, op1=OP.add)
    h = r
    v.memset(h[:], 0.0)
    for c in (1.0 / 720, 1.0 / 120, 1.0 / 24, 1.0 / 6, 0.5, 1.0):
        v.scalar_tensor_tensor(out=h[:], in0=h[:], scalar=float(c), in1=rr[:],
                               op0=OP.add, op1=OP.mult)
    v.tensor_scalar(out=h[:], in0=h[:], scalar1=1.0, scalar2=None, op0=OP.add)
    v.tensor_copy(ki[:], kf[:])
    v.tensor_scalar(out=ki[:], in0=ki[:], scalar1=127, scalar2=None, op0=OP.add)
    v.tensor_scalar(out=ki[:], in0=ki[:], scalar1=23, scalar2=None,
                    op0=OP.logical_shift_left)
    v.tensor_mul(out, h[:], ki[:].bitcast(F32))


def _patch_excess_waits(data: bytes) -> bytes:
    """walrus allows only ONE sync-wait command per instruction; move excess
    waits onto injected same-engine NoOps placed just before the offender."""
    d = json.loads(data)
    counter = [0]

    def fix_block(b):
        newlist = []
        for ins in b.get("instructions", []):
            si = ins.get("sync_info")
            ow = (si or {}).get("on_wait") or []
            if len(ow) > 1 and ins.get("engine") not in (None, "Unassigned"):
                for w in ow[:-1]:
                    newlist.append({
                        "debug": ins.get("debug", 0), "engine": ins["engine"],
                        "ins": [], "outs": [], "name": f"I-wsh{counter[0]}",
                        "opcode": "NoOp", "text_hint": "waitshield",
                        "sync_info": {"on_wait": [w], "on_update": []},
                    })
                    counter[0] += 1
                si["on_wait"] = [ow[-1]]
            newlist.append(ins)
        b["instructions"] = newlist
        for sub in b.get("blocks", []):
            fix_block(sub)

    for f in d["functions"]:
        blocks = f["blocks"]
        if isinstance(blocks, dict):
            blocks = [blocks]
        for blk in blocks:
            fix_block(blk)
    return json.dumps(d).encode()


def build_nc():
    nc = bass.Bass("TRN2", target_bir_lowering=False, debug=False, num_devices=NCORES)

    # ---------------- I/O ----------------
    xs_d = nc.dram_tensor("xs", (B * NS, D), F32, kind="ExternalInput")
    wah_d = nc.dram_tensor("wah", (128, 4, H), BF16, kind="ExternalInput")   # [p, dc, h] hi
    wal_d = nc.dram_tensor("wal", (128, 4, H), BF16, kind="ExternalInput")   # [p, dc, h] lo
    ba_d = nc.dram_tensor("ba", (128, 2), F32, kind="ExternalInput")         # [p, hc]
    wbr_d = nc.dram_tensor("wbr", (128, 2, C), F32, kind="ExternalInput")    # [p, hc, c]
    bbr_d = nc.dram_tensor("bbr", (C, 1), F32, kind="ExternalInput")
    w1_d = nc.dram_tensor("w1", (128, 2 * 4, H), F32, kind="ExternalInput")  # [p, c*4+dc, h]
    b1_d = nc.dram_tensor("b1", (128, 4), F32, kind="ExternalInput")         # [p, c*2+hc]
    w2_d = nc.dram_tensor("w2", (128, 4, 2), F32, kind="ExternalInput")      # [p, c*2+hc, o]
    b2_d = nc.dram_tensor("b2", (2, C), F32, kind="ExternalInput")           # [o, c]
    ident_d = nc.dram_tensor("ident", (128, 128), F32, kind="ExternalInput")
    ones1_d = nc.dram_tensor("ones1", (1, 128), F32, kind="ExternalInput")
    onesc_d = nc.dram_tensor("onesc", (128, 1), F32, kind="ExternalInput")
    onesm_d = nc.dram_tensor("onesm", (128, 128), BF16, kind="ExternalInput")
    ltri_d = nc.dram_tensor("ltri", (16, 16), F32, kind="ExternalInput")     # [a,p]=1 iff a<p
    boff_d = nc.dram_tensor("boff", (16, 4), F32, kind="ExternalInput")      # b*2048
    cbase_d = nc.dram_tensor("cbase", (128, 1), F32, kind="ExternalInput")   # core_id*2048
    brow_d = nc.dram_tensor("brow", (128, 1), F32, kind="ExternalInput")     # b(r)*2048
    lab_d = nc.dram_tensor("lab", (64, 1), I32, kind="ExternalInput")        # label per inst col
    bco_d = nc.dram_tensor("bco", (8, 1), I32, kind="ExternalInput")         # bc*8
    pidx_d = nc.dram_tensor("pidx", (128, 1), F32, kind="ExternalInput")     # p*128
    bcof_d = nc.dram_tensor("bcof", (8, 1), F32, kind="ExternalInput")       # bc*8 f32
    qoff_d = nc.dram_tensor("qoff", (16, 1), I32, kind="ExternalInput")      # i*16+q

    comb_o = nc.dram_tensor("out_comb", (B, N), F32, kind="ExternalOutput")
    thr_o = nc.dram_tensor("out_thr", (1, 4), F32, kind="ExternalOutput")
    sel_os = [nc.dram_tensor(f"out_sel{b}", (NS, D), F32, kind="ExternalOutput")
              for b in range(B)]
    cl_o = nc.dram_tensor("out_cl", (1, 1), F32, kind="ExternalOutput")
    cnt_o = nc.dram_tensor("out_cnt", (1, 4), F32, kind="ExternalOutput")

    with TileContext(nc) as tc:
        with (
            tc.tile_pool(name="const", bufs=1) as cp,
            tc.tile_pool(name="work", bufs=1) as wp,
            tc.tile_pool(name="dram", bufs=1, space="DRAM") as dr,
            tc.tile_pool(name="psB", bufs=2, space="PSUM") as psB,
        ):
            # ------------- constant loads -------------
            wah = cp.tile([128, 4, H], BF16, name="wah")
            nc.sync.dma_start(wah[:], wah_d.ap())
            wal = cp.tile([128, 4, H], BF16, name="wal")
            nc.sync.dma_start(wal[:], wal_d.ap())
            ba = cp.tile([128, 2], F32, name="ba")
            nc.sync.dma_start(ba[:], ba_d.ap())
            wbr = cp.tile([128, 2, C], F32, name="wbr")
            nc.sync.dma_start(wbr[:], wbr_d.ap())
            bbr = cp.tile([C, 1], F32, name="bbr")
            nc.sync.dma_start(bbr[:], bbr_d.ap())
            w1 = cp.tile([128, 8, H], F32, name="w1")
            nc.sync.dma_start(w1[:], w1_d.ap())
            b1 = cp.tile([128, 4], F32, name="b1")
            nc.sync.dma_start(b1[:], b1_d.ap())
            w2 = cp.tile([128, 4, 2], F32, name="w2")
            nc.sync.dma_start(w2[:], w2_d.ap())
            b2 = cp.tile([2, C], F32, name="b2")
            nc.sync.dma_start(b2[:], b2_d.ap())
            ident = cp.tile([128, 128], F32, name="ident")
            nc.sync.dma_start(ident[:], ident_d.ap())
            ones1 = cp.tile([1, 128], F32, name="ones1")
            nc.sync.dma_start(ones1[:], ones1_d.ap())
            onesc = cp.tile([128, 1], F32, name="onesc")
            nc.sync.dma_start(onesc[:], onesc_d.ap())
            onesm = cp.tile([128, 128], BF16, name="onesm")
            nc.sync.dma_start(onesm[:], onesm_d.ap())
            ltri = cp.tile([16, 16], F32, name="ltri")
            nc.sync.dma_start(ltri[:], ltri_d.ap())
            boff = cp.tile([16, 4], F32, name="boff")
            nc.sync.dma_start(boff[:], boff_d.ap())
            cbase = cp.tile([128, 1], F32, name="cbase")
            nc.sync.dma_start(cbase[:], cbase_d.ap())
            brow = cp.tile([128, 1], F32, name="brow")
            nc.sync.dma_start(brow[:], brow_d.ap())
            lab = cp.tile([64, 1], I32, name="lab")
            nc.sync.dma_start(lab[:], lab_d.ap())
            bco = cp.tile([8, 1], I32, name="bco")
            nc.sync.dma_start(bco[:], bco_d.ap())
            pidx = cp.tile([128, 1], F32, name="pidx")
            nc.sync.dma_start(pidx[:], pidx_d.ap())
            bcof = cp.tile([8, 1], F32, name="bcof")
            nc.sync.dma_start(bcof[:], bcof_d.ap())
            qoff = cp.tile([16, 1], I32, name="qoff")
            nc.sync.dma_start(qoff[:], qoff_d.ap())

            # Lsb and Lg (later) share one 64KB/partition slot via tag
            Lsb = wp.tile([C, B, NS], F32, name="Lsb", tag="bigslot")  # [c, b, n]
            # allocate the indirect-DMA bounds register before collectives
            # grab gpsimd's register file
            bc_reg = nc.gpsimd.to_reg(B * NS - 1)
            bc_reg_ns = nc.gpsimd.to_reg(NS - 1)
            exp_scr = {
                "t": wp.tile([128, 1024], F32, name="exp_t"),
                "kf": wp.tile([128, 1024], F32, name="exp_kf"),
                "r": wp.tile([128, 1024], F32, name="exp_r"),
                "rr": wp.tile([128, 1024], F32, name="exp_rr"),
                "ki": wp.tile([128, 1024], I32, name="exp_ki"),
            }

            # dram scratch
            cc_in = dr.tile([C, B, NS], F32, name="cc_in")
            cc_out = dr.tile([NCORES, C, B, NS], F32, name="cc_out")
            ci_in = dr.tile([128, D], F32, name="ci_in")
            ci_out = dr.tile([128, D], F32, name="ci_out")
            cvd = dr.tile([128, 64], F32, name="cvd")
            cvnd = dr.tile([128, 64], F32, name="cvnd")
            qd2 = dr.tile([128, 1], I32, name="qd2")      # flat positions bounce
            cboth = dr.tile([256, 64], I32, name="cboth") # candN (top) ++ candNn (bot)
            maskd = dr.tile([128, 4, 128], F32, name="maskd")  # global mask bounce

            # ============ Phase A: GEMM over 16 chunks of 512 rows ============
            # X -> bf16 hi/lo split on DVE, DMA-transpose (xbar) to get
            # contraction dim on partitions, 3-pass bf16 matmul (hi*hi +
            # hi*lo + lo*hi) accumulated in fp32 PSUM.
            xs_flat = xs_d.ap()  # (8192, 512)
            with (
                tc.tile_pool(name="xp", bufs=2) as xp,
                tc.tile_pool(name="xtp", bufs=2) as xtp,
                tc.tile_pool(name="htp", bufs=3) as htp,
                tc.tile_pool(name="psA", bufs=1, space="PSUM") as psA,
            ):
                for ch in range(16):
                    b_i, u = ch // 4, ch % 4
                    Xc = xp.tile([128, 4, D], F32, name="Xc", tag="Xc", bufs=2)
                    src = xs_flat[ch * 512:(ch + 1) * 512, :].rearrange(
                        "(s p) d -> p s d", p=128)
                    nc.sync.dma_start(Xc[:], src)
                    Xhi = xp.tile([128, 4, D], BF16, name="Xhi", tag="Xhi", bufs=3)
                    nc.vector.tensor_copy(Xhi[:], Xc[:])
                    Xlo = xp.tile([128, 4, D], BF16, name="Xlo", tag="Xlo", bufs=3)
                    nc.vector.tensor_tensor(out=Xlo[:], in0=Xc[:], in1=Xhi[:],
                                            op=OP.subtract)
                    # one xbar transpose per operand: out[p, e, c] = in[c, e*128+p]
                    # with in free f = s*512+d  ->  e = s*4+dc, p = d%128
                    XTh = xtp.tile([128, 16, 128], BF16, name="XTh", tag="XTh", bufs=4)
                    XTl = xtp.tile([128, 16, 128], BF16, name="XTl", tag="XTl", bufs=4)
                    nc.sync.dma_start_transpose(
                        XTh[:], Xhi[:].rearrange("p s d -> p (s d)"))
                    nc.sync.dma_start_transpose(
                        XTl[:], Xlo[:].rearrange("p s d -> p (s d)"))
                    aps = psA.tile([C, 512], F32, name="aps", tag="aps", bufs=2)
                    for hc in range(2):
                        hps = psA.tile([128, 512], F32, name="hps", tag="hps", bufs=2)
                        first = True
                        for dc in range(4):
                            for lhs, rhs in ((wah, XTh), (wah, XTl), (wal, XTh)):
                                nc.tensor.matmul(
                                    out=hps[:], lhsT=lhs[:, dc, hc * 128:(hc + 1) * 128],
                                    rhs=rhs[:, dc::4, :],
                                    start=first, stop=(dc == 3 and lhs is wal))
                                first = False
                        hT = htp.tile([128, 512], F32, name="hT", tag="hT", bufs=3)
                        nc.scalar.activation(hT[:], hps[:], ACTF.Relu, bias=ba[:, hc:hc + 1])
                        nc.tensor.matmul(out=aps[:], lhsT=wbr[:, hc, :], rhs=hT[:],
                                         start=(hc == 0), stop=(hc == 1))
                    nc.vector.tensor_scalar(out=Lsb[:, b_i, u * 512:(u + 1) * 512],
                                            in0=aps[:], scalar1=bbr[:, :1], scalar2=None,
                                            op0=OP.add)

            # ============ Phase B: AllGather logits ============
            nc.sync.dma_start(cc_in[:], Lsb[:])
            nc.gpsimd.collective_compute(
                "AllGather", OP.bypass, replica_groups=[list(range(NCORES))],
                ins=[cc_in.opt()], outs=[cc_out.opt()],
            )

            # ============ Phase C: global softmax / combined ============
            Lt = wp.tile([128, 8, 128], F32, name="Lt")
            for r in range(NCORES):
                nc.sync.dma_start(
                    Lt[r * 16:(r + 1) * 16, :, :],
                    cc_out[r].rearrange("c b (q j) -> q (c b) j", j=128))
            Et = wp.tile([128, 8, 128], F32, name="Et")
            _emit_exp(nc, exp_scr, Et[:].rearrange("p a b -> p (a b)"),
                      Lt[:].rearrange("p a b -> p (a b)"), 128)
            partial = wp.tile([128, 8], F32, name="partial")
            nc.vector.tensor_reduce(out=partial[:], in_=Et[:], axis=AX.X, op=OP.add)
            tpp = psB.tile([8, 128], F32, name="tpp", tag="ps")
            nc.tensor.transpose(tpp[:], partial[:], ident[:])
            pt_sb = wp.tile([8, 128], F32, name="pt_sb")
            nc.vector.tensor_copy(pt_sb[:], tpp[:])
            S = wp.tile([8, 1], F32, name="S")
            nc.vector.tensor_reduce(out=S[:], in_=pt_sb[:], axis=AX.X, op=OP.add)
            invS = wp.tile([8, 1], F32, name="invS")
            nc.vector.reciprocal(invS[:], S[:])
            tiv = psB.tile([1, 8], F32, name="tiv", tag="ps")
            nc.tensor.transpose(tiv[:], invS[:], ident[:8, :8])
            iv_sb = wp.tile([1, 8], F32, name="iv_sb")
            nc.vector.tensor_copy(iv_sb[:], tiv[:])
            pib = psB.tile([128, 8], F32, name="pib", tag="ps")
            nc.tensor.matmul(out=pib[:], lhsT=ones1[:], rhs=iv_sb[:], start=True, stop=True)
            invB = wp.tile([128, 8], F32, name="invB")
            nc.vector.tensor_copy(invB[:], pib[:])
            # P = E * invS, in place over Et
            nc.vector.tensor_mul(Et[:], Et[:], invB[:, :, None].to_broadcast([128, 8, 128]))
            comb = wp.tile([128, 4, 128], F32, name="comb")
            nc.vector.tensor_add(comb[:], Et[:, 0:4, :], Et[:, 4:8, :])
            nc.vector.tensor_scalar(out=comb[:], in0=comb[:], scalar1=0.5, scalar2=None,
                                    op0=OP.mult)

            # ============ Phase D: binary search for per-b thresholds ============
            m1 = wp.tile([128, 4], F32, name="m1")
            nc.vector.tensor_reduce(out=m1[:], in_=comb[:], axis=AX.X, op=OP.max)
            tm1 = psB.tile([4, 128], F32, name="tm1", tag="ps")
            nc.tensor.transpose(tm1[:], m1[:], ident[:])
            tm1_sb = wp.tile([4, 128], F32, name="tm1_sb")
            nc.vector.tensor_copy(tm1_sb[:], tm1[:])
            m2 = wp.tile([4, 1], F32, name="m2")
            nc.vector.tensor_reduce(out=m2[:], in_=tm1_sb[:], axis=AX.X, op=OP.max)
            tm2 = psB.tile([1, 4], F32, name="tm2", tag="ps")
            nc.tensor.transpose(tm2[:], m2[:], ident[:4, :4])
            hi14 = wp.tile([1, 4], F32, name="hi14")
            nc.vector.tensor_copy(hi14[:], tm2[:])
            pib2 = psB.tile([128, 4], F32, name="pib2", tag="ps")
            nc.tensor.matmul(out=pib2[:], lhsT=ones1[:], rhs=hi14[:], start=True, stop=True)
            hi = wp.tile([128, 4], F32, name="hi")
            nc.vector.tensor_copy(hi[:], pib2[:])
            lo = wp.tile([128, 4], F32, name="lo")
            nc.vector.memset(lo[:], 0.0)
            mid = wp.tile([128, 4], F32, name="mid")
            gts = wp.tile([128, 4, 128], F32, name="gts")
            cnt128 = wp.tile([128, 4], BF16, name="cnt128")
            ge = wp.tile([128, 4], U32, name="ge")
            lt = wp.tile([128, 4], U32, name="lt")
            for it in range(SEARCH_ITERS):
                nc.vector.tensor_add(mid[:], lo[:], hi[:])
                nc.vector.tensor_scalar(out=mid[:], in0=mid[:], scalar1=0.5, scalar2=None,
                                        op0=OP.mult)
                for b_i in range(4):
                    nc.vector.tensor_scalar(out=gts[:, b_i, :], in0=comb[:, b_i, :],
                                            scalar1=mid[:, b_i:b_i + 1], scalar2=None,
                                            op0=OP.is_gt)
                with nc.allow_low_precision("counts <= 128 are exact in bf16"):
                    nc.vector.tensor_reduce(out=cnt128[:], in_=gts[:], axis=AX.X, op=OP.add)
                # replicated total: every partition gets sum over partitions
                pcn = psB.tile([128, 4], F32, name="pcn", tag="ps")
                nc.tensor.matmul(out=pcn[:], lhsT=onesm[:], rhs=cnt128[:], start=True, stop=True)
                nc.vector.tensor_scalar(out=ge[:], in0=pcn[:], scalar1=float(K),
                                        scalar2=None, op0=OP.is_ge)
                nc.vector.tensor_scalar(out=lt[:], in0=pcn[:], scalar1=float(K),
                                        scalar2=None, op0=OP.is_lt)
                nc.vector.copy_predicated(lo[:], ge[:], mid[:])
                nc.vector.copy_predicated(hi[:], lt[:], mid[:])
            nc.sync.dma_start(thr_o.ap(), lo[0:1, :])
            # final count (debug/assert)
            thr128 = lo
            for b_i in range(4):
                nc.vector.tensor_scalar(out=gts[:, b_i, :], in0=comb[:, b_i, :],
                                        scalar1=thr128[:, b_i:b_i + 1], scalar2=None,
                                        op0=OP.is_gt)
            with nc.allow_low_precision("counts <= 128 are exact in bf16"):
                nc.vector.tensor_reduce(out=cnt128[:], in_=gts[:], axis=AX.X, op=OP.add)
            pcf = psB.tile([128, 4], F32, name="pcf", tag="ps")
            nc.tensor.matmul(out=pcf[:], lhsT=onesm[:], rhs=cnt128[:], start=True, stop=True)
            cntf = wp.tile([1, 4], F32, name="cntf")
            nc.vector.tensor_copy(cntf[:], pcf[0:1, :])
            nc.sync.dma_start(cnt_o.ap(), cntf[:])
            nc.sync.dma_start(maskd[:], gts[:])
            # write global combined
            nc.sync.dma_start(
                comb_o.ap().rearrange("b (p j) -> p b j", p=128), comb[:])

            # ============ Phase E: local compaction ============
            # gather this core's 16 partition-rows of the global mask from the
            # dram bounce (row offsets are a per-core input -> static code,
            # and the mask is by construction identical to the host's
            # comb > thr comparison)
            maskL = wp.tile([16, 4, 128], F32, name="maskL")
            nc.gpsimd.indirect_dma_start(
                out=maskL[:].rearrange("q a b -> q (a b)"),
                out_offset=None,
                in_=maskd[:].rearrange("p a b -> p (a b)"),
                in_offset=bass.IndirectOffsetOnAxis(ap=qoff[:, :1], axis=0),
                bounds_check=None, oob_is_err=True)
            zeros16 = wp.tile([16, 128], F32, name="zeros16")
            nc.vector.memset(zeros16[:], 0.0)
            pref = wp.tile([16, 4, 128], F32, name="pref")
            for b_i in range(4):
                nc.vector.tensor_tensor_scan(out=pref[:, b_i, :], data0=maskL[:, b_i, :],
                                             data1=zeros16[:], initial=0.0,
                                             op0=OP.add, op1=OP.add)
            Ops = psB.tile([16, 4], F32, name="Ops", tag="ps")
            nc.tensor.matmul(out=Ops[:], lhsT=ltri[:], rhs=pref[:, :, 127], start=True, stop=True)
            obb = wp.tile([16, 4], F32, name="obb")
            nc.vector.tensor_add(obb[:], Ops[:], boff[:])
            pos = wp.tile([16, 4, 128], F32, name="pos")
            nc.vector.tensor_sub(pos[:], pref[:], maskL[:])
            nc.vector.tensor_add(pos[:], pos[:], obb[:, :, None].to_broadcast([16, 4, 128]))
            bigsel = wp.tile([16, 4, 128], F32, name="bigsel")
            nc.vector.memset(bigsel[:], BIG)
            maskLi = wp.tile([16, 4, 128], U32, name="maskLi")
            nc.vector.tensor_copy(maskLi[:], maskL[:])
            posx = wp.tile([16, 4, 128], F32, name="posx")
            nc.vector.select(posx[:], maskLi[:], pos[:], bigsel[:])
            post = wp.tile([128, 4, 16], I32, name="post")
            for b_i in range(4):
                ptp = psB.tile([128, 16], F32, name="ptp", tag="ps")
                nc.tensor.transpose(ptp[:], posx[:, b_i, :], ident[:16, :16])
                nc.vector.tensor_copy(post[:, b_i, :], ptp[:])

            # ============ Phase F: masked scatter of selected rows ============
            with tc.tile_pool(name="fp", bufs=24) as fp:
                for t in range(64):
                    b_i, q = t % 4, t // 4   # interleave b so the 4 WAW chains pipeline
                    tt = b_i * 16 + q
                    Xs = fp.tile([128, D], F32, name="Xs", tag="Xs", bufs=24)
                    nc.sync.dma_start(Xs[:], xs_flat[tt * 128:(tt + 1) * 128, :])
                    nc.gpsimd.indirect_dma_start(
                        out=sel_os[b_i].ap(),
                        out_offset=bass.IndirectOffsetOnAxis(
                            ap=post[:, b_i, q:q + 1], axis=0),
                        in_=Xs[:],
                        in_offset=None,
                        bounds_check=bc_reg_ns,
                        oob_is_err=False,
                    )

            # ============ Phase G: instance top8/bot8 + gather ============
            # hierarchical: per-partition top8 candidates on the (128, 8, 128)
            # layout, global top8 over the 1024 candidates, index recovery via
            # a small DRAM bounce + indirect gather of the candidate ids.
            candV = wp.tile([128, 8, 8], F32, name="candV")
            candJ = wp.tile([128, 8, 8], U32, name="candJ")
            candN = wp.tile([128, 8, 8], I32, name="candN")
            candVn = wp.tile([128, 8, 8], F32, name="candVn")
            candJn = wp.tile([128, 8, 8], U32, name="candJn")
            candNn = wp.tile([128, 8, 8], I32, name="candNn")
            for bc in range(8):
                nc.vector.max(out=candV[:, bc, :], in_=Lt[:, bc, :])
                nc.vector.max_index(out=candJ[:, bc, :], in_max=candV[:, bc, :],
                                    in_values=Lt[:, bc, :])
            candNf = wp.tile([128, 8, 8], F32, name="candNf")
            nc.vector.tensor_scalar(out=candNf[:].rearrange("p a b -> p (a b)"),
                                    in0=candJ[:].rearrange("p a b -> p (a b)"),
                                    scalar1=pidx[:, :1], scalar2=None, op0=OP.add)
            nc.vector.tensor_copy(candN[:].rearrange("p a b -> p (a b)"),
                                  candNf[:].rearrange("p a b -> p (a b)"))
            nc.vector.tensor_scalar(out=Lt[:].rearrange("p a b -> p (a b)"),
                                    in0=Lt[:].rearrange("p a b -> p (a b)"),
                                    scalar1=-1.0, scalar2=None, op0=OP.mult)
            for bc in range(8):
                nc.vector.max(out=candVn[:, bc, :], in_=Lt[:, bc, :])
                nc.vector.max_index(out=candJn[:, bc, :],
                                    in_max=candVn[:, bc, :], in_values=Lt[:, bc, :])
            nc.vector.tensor_scalar(out=candNf[:].rearrange("p a b -> p (a b)"),
                                    in0=candJn[:].rearrange("p a b -> p (a b)"),
                                    scalar1=pidx[:, :1], scalar2=None, op0=OP.add)
            nc.vector.tensor_copy(candNn[:].rearrange("p a b -> p (a b)"),
                                  candNf[:].rearrange("p a b -> p (a b)"))
            nc.sync.dma_start(cvd[:], candV[:].rearrange("p a b -> p (a b)"))
            nc.sync.dma_start(cboth[0:128, :], candN[:].rearrange("p a b -> p (a b)"))
            nc.sync.dma_start(cvnd[:], candVn[:].rearrange("p a b -> p (a b)"))
            nc.sync.dma_start(cboth[128:256, :], candNn[:].rearrange("p a b -> p (a b)"))
            candVg = wp.tile([8, 128, 8], F32, name="candVg")
            candVgn = wp.tile([8, 128, 8], F32, name="candVgn")
            nc.sync.dma_start(candVg[:], cvd[:].rearrange("p (a r) -> a p r", a=8))
            nc.sync.dma_start(candVgn[:], cvnd[:].rearrange("p (a r) -> a p r", a=8))
            gtop = wp.tile([8, 8], F32, name="gtop")
            gq = wp.tile([8, 8], U32, name="gq")
            nc.vector.max(out=gtop[:], in_=candVg[:])
            nc.vector.max_index(out=gq[:], in_max=gtop[:],
                                in_values=candVg[:].rearrange("a p r -> a (p r)"))
            gtopn = wp.tile([8, 8], F32, name="gtopn")
            gqn = wp.tile([8, 8], U32, name="gqn")
            nc.vector.max(out=gtopn[:], in_=candVgn[:])
            nc.vector.max_index(out=gqn[:], in_max=gtopn[:],
                                in_values=candVgn[:].rearrange("a p r -> a (p r)"))
            # flat position in the cnd dram bounce: (q>>3)*64 + bc*8 + (q&7)
            flats = wp.tile([8, 2, 8], I32, name="flats")
            for src_q, tb in ((gq, 0), (gqn, 1)):
                f1 = wp.tile([8, 8], U32, name=f"f1_{tb}")
                nc.vector.tensor_scalar(out=f1[:], in0=src_q[:], scalar1=3,
                                        scalar2=6, op0=OP.logical_shift_right,
                                        op1=OP.logical_shift_left)
                f2 = wp.tile([8, 8], U32, name=f"f2_{tb}")
                nc.vector.tensor_scalar(out=f2[:], in0=src_q[:], scalar1=7,
                                        scalar2=None, op0=OP.bitwise_and)
                nc.vector.tensor_add(f1[:], f1[:], f2[:])
                ff = wp.tile([8, 8], F32, name=f"ff_{tb}")
                nc.vector.tensor_scalar(out=ff[:], in0=f1[:],
                                        scalar1=bcof[:, :1],
                                        scalar2=(128.0 * 64 if tb == 1 else 0.0),
                                        op0=OP.add, op1=OP.add)
                nc.vector.tensor_copy(flats[:, tb, :], ff[:])
            nc.sync.dma_start(qd2[:].rearrange("(a k) o -> a (k o)", a=8),
                              flats[:].rearrange("a b c -> a (b c)"))
            flat128 = wp.tile([128, 1], I32, name="flat128")
            nc.sync.dma_start(flat128[:], qd2[:])
            ng128 = wp.tile([128, 1], I32, name="ng128")
            nc.gpsimd.indirect_dma_start(
                out=ng128[:], out_offset=None,
                in_=cboth[:].rearrange("p k -> (p k)")[:, None],
                in_offset=bass.IndirectOffsetOnAxis(ap=flat128[:, :1], axis=0),
                bounds_check=None, oob_is_err=True)
            ngf = wp.tile([128, 1], F32, name="ngf")
            nc.vector.tensor_copy(ngf[:], ng128[:])
            t0 = wp.tile([128, 1], F32, name="t0")
            nc.vector.tensor_scalar(out=t0[:], in0=ngf[:], scalar1=cbase[:, :1],
                                    scalar2=None, op0=OP.subtract)
            v1 = wp.tile([128, 1], U32, name="v1")
            nc.vector.tensor_scalar(out=v1[:], in0=t0[:], scalar1=0.0, scalar2=None,
                                    op0=OP.is_ge)
            v2 = wp.tile([128, 1], U32, name="v2")
            nc.vector.tensor_scalar(out=v2[:], in0=t0[:], scalar1=float(NS), scalar2=None,
                                    op0=OP.is_lt)
            nc.vector.tensor_mul(v1[:], v1[:], v2[:])
            ladd = wp.tile([128, 1], F32, name="ladd")
            nc.vector.tensor_add(ladd[:], t0[:], brow[:])
            big1 = wp.tile([128, 1], F32, name="big1")
            nc.vector.memset(big1[:], BIG)
            lidxf = wp.tile([128, 1], F32, name="lidxf")
            nc.vector.select(lidxf[:], v1[:], ladd[:], big1[:])
            lidx = wp.tile([128, 1], I32, name="lidx")
            nc.vector.tensor_copy(lidx[:], lidxf[:])
            inst = wp.tile([128, D], F32, name="inst")
            nc.vector.memset(inst[:], 0.0)
            nc.gpsimd.indirect_dma_start(
                out=inst[:], out_offset=None,
                in_=xs_d.ap(),
                in_offset=bass.IndirectOffsetOnAxis(ap=lidx[:, :1], axis=0),
                bounds_check=bc_reg, oob_is_err=False,
            )
            nc.sync.dma_start(ci_in[:], inst[:])
            nc.gpsimd.collective_compute(
                "AllReduce", OP.add, replica_groups=[list(range(NCORES))],
                ins=[ci_in.opt()], outs=[ci_out.opt()],
            )
            inst2 = wp.tile([128, D], F32, name="inst2")
            nc.sync.dma_start(inst2[:], ci_out[:])

            # ============ Phase H: instance MLP + CE ============
            instT = wp.tile([128, 4, 128], F32, name="instT")
            for dc in range(4):
                itp = psB.tile([128, 128], F32, name="itp", tag="ps")
                nc.tensor.transpose(itp[:], inst2[:, dc * 128:(dc + 1) * 128], ident[:])
                nc.scalar.copy(instT[:, dc, :], itp[:])
            clp = psB.tile([1, 1], F32, name="clp", tag="clp", bufs=1)
            for c_i in range(2):
                h1s = []
                for hc in range(2):
                    hp1 = psB.tile([128, 64], F32, name="hp1", tag="ps")
                    for dc in range(4):
                        nc.tensor.matmul(out=hp1[:], lhsT=w1[:, c_i * 4 + dc, hc * 128:(hc + 1) * 128],
                                         rhs=instT[:, dc, c_i * 64:(c_i + 1) * 64],
                                         start=(dc == 0), stop=(dc == 3))
                    h1sb = wp.tile([128, 64], F32, name=f"h1sb{c_i}{hc}")
                    nc.scalar.activation(h1sb[:], hp1[:], ACTF.Relu,
                                         bias=b1[:, c_i * 2 + hc:c_i * 2 + hc + 1])
                    h1s.append(h1sb)
                l2p = psB.tile([2, 64], F32, name="l2p", tag="ps")
                for hc in range(2):
                    nc.tensor.matmul(out=l2p[:], lhsT=w2[:, c_i * 2 + hc, :], rhs=h1s[hc][:],
                                     start=(hc == 0), stop=(hc == 1))
                l2s = wp.tile([2, 64], F32, name=f"l2s{c_i}")
                nc.vector.tensor_scalar(out=l2s[:], in0=l2p[:], scalar1=b2[:, c_i:c_i + 1],
                                        scalar2=None, op0=OP.add)
                l2tp = psB.tile([64, 2], F32, name="l2tp", tag="ps")
                nc.tensor.transpose(l2tp[:], l2s[:], ident[:2, :2])
                l2T = wp.tile([64, 2], F32, name=f"l2T{c_i}")
                nc.vector.tensor_copy(l2T[:], l2tp[:])
                mx = wp.tile([64, 1], F32, name=f"mx{c_i}")
                nc.vector.tensor_reduce(out=mx[:], in_=l2T[:], axis=AX.X, op=OP.max)
                mneg = wp.tile([64, 1], F32, name=f"mneg{c_i}")
                nc.vector.tensor_scalar(out=mneg[:], in0=mx[:], scalar1=-1.0, scalar2=None,
                                        op0=OP.mult)
                ex = wp.tile([64, 2], F32, name=f"ex{c_i}")
                nc.scalar.activation(ex[:], l2T[:], ACTF.Exp, bias=mneg[:, :1])
                sm = wp.tile([64, 1], F32, name=f"sm{c_i}")
                nc.vector.tensor_reduce(out=sm[:], in_=ex[:], axis=AX.X, op=OP.add)
                lse = wp.tile([64, 1], F32, name=f"lse{c_i}")
                nc.scalar.activation(lse[:], sm[:], ACTF.Ln)
                lsel = wp.tile([64, 1], F32, name=f"lsel{c_i}")
                nc.vector.select(lsel[:], lab[:], l2T[:, 1:2], l2T[:, 0:1])
                ce = wp.tile([64, 1], F32, name=f"ce{c_i}")
                nc.vector.tensor_add(ce[:], mx[:], lse[:])
                nc.vector.tensor_sub(ce[:], ce[:], lsel[:])
                nc.tensor.matmul(out=clp[:], lhsT=ce[:], rhs=onesc[:64, :],
                                 start=(c_i == 0), stop=(c_i == 1))
            cls = wp.tile([1, 1], F32, name="cls")
            nc.scalar.activation(cls[:], clp[:], ACTF.Copy, scale=1.0 / 128)
            nc.sync.dma_start(cl_o.ap(), cls[:])

    return nc


_NC_CACHE = None


def _get_nc():
    global _NC_CACHE
    if _NC_CACHE is None:
        nc = build_nc()
        orig = nc.to_json_bytes
        nc.to_json_bytes = lambda: _patch_excess_waits(orig())
        _NC_CACHE = nc
    return _NC_CACHE


def _host_inputs(features, Wa, ba, Wbr, bbr, W1, b1, W2, b2):
    """Build the per-core input maps."""
    import ml_dtypes
    _wa = Wa.reshape(4, 128, H).transpose(1, 0, 2).copy()
    _wa_hi = _wa.astype(ml_dtypes.bfloat16)
    _wa_lo = (_wa - _wa_hi.astype(np.float32)).astype(ml_dtypes.bfloat16)
    common = {
        "wah": _wa_hi,
        "wal": _wa_lo,
        "ba": ba.reshape(2, 128).T.copy(),
        "wbr": Wbr.T.reshape(2, 128, C).transpose(1, 0, 2).copy(),
        "bbr": bbr[:, None].copy(),
        "w1": W1.reshape(C * 4, 128, H).transpose(1, 0, 2).copy(),
        "b1": b1.reshape(C, 2, 128).transpose(2, 0, 1).reshape(128, 4).copy(),
        "w2": W2.reshape(C * 2, 128, 2).transpose(1, 0, 2).copy(),
        "b2": b2.T.copy(),
        "ident": np.eye(128, dtype=np.float32),
        "ones1": np.ones((1, 128), dtype=np.float32),
        "onesc": np.ones((128, 1), dtype=np.float32),
        "onesm": np.ones((128, 128), dtype=ml_dtypes.bfloat16),
        "ltri": np.triu(np.ones((16, 16), dtype=np.float32), k=1),
        "boff": np.zeros((16, 4), dtype=np.float32),
        "brow": (np.repeat(np.arange(8) % 4, 16).astype(np.float32) * NS)[:, None],
        "lab": np.tile(np.r_[np.ones(8), np.zeros(8)].astype(np.int32), 4)[:, None],
        "bco": (np.arange(8, dtype=np.int32) * 8)[:, None],
        "pidx": (np.arange(128, dtype=np.float32) * 128)[:, None],
        "bcof": (np.arange(8, dtype=np.float32) * 8)[:, None],
    }
    in_maps = []
    for i in range(NCORES):
        m = dict(common)
        m["xs"] = np.ascontiguousarray(
            features[:, i * NS:(i + 1) * NS, :].reshape(B * NS, D))
        m["cbase"] = np.full((128, 1), i * NS, dtype=np.float32)
        m["qoff"] = (np.arange(16, dtype=np.int32) + i * 16)[:, None]
        in_maps.append(m)
    return in_maps


def kernel(features, Wa, ba, Wbr, bbr, W1, b1, W2, b2):
    features = np.asarray(features, dtype=np.float32)
    in_maps = _host_inputs(features, np.asarray(Wa), np.asarray(ba), np.asarray(Wbr),
                           np.asarray(bbr), np.asarray(W1), np.asarray(b1),
                           np.asarray(W2), np.asarray(b2))
    nc = _get_nc()
    trace = bool(int(os.environ.get("KERNEL_TRACE", "0")))
    if trace:
        try:
            from antenv.axon_hooks import set_axon_ntff_profile_hook
            from trn_agent_boot.trn_boot import _ntff_profile_via_ctypes
            set_axon_ntff_profile_hook(_ntff_profile_via_ctypes("/opt/axon/libaxon_pjrt.so"))
        except Exception as e:
            print("ntff hook setup failed:", e)
    res = run_bass_kernel_spmd(nc, in_maps, core_ids=list(range(NCORES)), trace=trace)
    if trace:
        kernel.last_result = res
    r0 = res.results[0]
    comb = r0["out_comb"]                      # (4, 16384)
    thr = r0["out_thr"][0]                     # (4,)
    mask = comb > thr[:, None]
    counts = mask.sum(axis=1)
    assert np.all(counts == K), f"threshold search failed: counts={counts}"
    sel_idx = np.zeros((B, K), dtype=np.int32)
    selected = np.empty((B, K, D), dtype=np.float32)
    for b_i in range(B):
        idx = np.nonzero(mask[b_i])[0]
        sel_idx[b_i] = idx.astype(np.int32)
        off = 0
        for i in range(NCORES):
            c = int(mask[b_i, i * NS:(i + 1) * NS].sum())
            rows = res.results[i][f"out_sel{b_i}"][:c, :]
            selected[b_i, off:off + c] = rows
            off += c
        assert off == K
    cl_loss = np.float32(r0["out_cl"][0, 0])
    return selected, comb, sel_idx, cl_loss


# revision 37
# speedup vs baseline: 1.0487x; 1.0487x over previous
"""CLAMSelector kernel for 8 TRN2 NeuronCores (Bass/Tile, SPMD).

Problem: B=4, N=16384, D=512, H=256, C=2; top-k (k=11468) selection over
combined attention + per-class instance-clustering loss.

Sharding: N split across 8 cores (2048 patches each). Per core:
  - fp32 GEMM  h = relu(X@Wa+ba), logits = h@Wbr^T+bbr for its shard
  - AllGather logits (64KB) -> global softmax (poly-exp on DVE, ~2ulp)
  - combined = mean over classes; branchless 28-iter binary search for the
    exact k-th threshold (all 4 batch rows in parallel)
  - local mask + prefix-scan compaction; masked indirect-DMA scatter writes
    only the selected feature rows (OOB positions skipped by bounds_check)
  - instance loss: global top8/bot8 per (b,c) via hierarchical max8/max_index,
    cross-shard row gather via bounds-checked indirect DMA + AllReduce,
    tiny fp32 MLP -> scalar loss
Host assembles full outputs from per-core compacted shards.
"""
import sys
import os

sys.path.insert(0, "/opt/trn_rl_repo")

import json
import numpy as np

import concourse.bass as bass
import concourse.mybir as mybir
from concourse.tile import TileContext
from concourse.bass_utils import run_bass_kernel_spmd

F32 = mybir.dt.float32
BF16 = mybir.dt.bfloat16
I32 = mybir.dt.int32
U32 = mybir.dt.uint32
OP = mybir.AluOpType
AX = mybir.AxisListType
ACTF = mybir.ActivationFunctionType

B, N, D, H, C = 4, 16384, 512, 256, 2
NCORES = 8
NS = N // NCORES          # 2048 patches per core
K = 11468                 # top-k (int(N*0.7))
KK = 8                    # instances per side
SEARCH_ITERS = 18
BIG = 1.0e7

# ---- poly exp constants (exp via 2^k * P(r), |r| <= ln2/2, ~2ulp) ----
LN2_HI = 0.693359375
LN2_LO = -2.12194440e-4
INV_LN2 = 1.4426950408889634
MAGIC = 12582912.0  # 1.5 * 2**23


def _emit_exp(nc, scratch, out, in_, npart):
    """out = exp(in_) elementwise on DVE, fp32 ~2ulp. Deterministic op
    sequence (identical per element regardless of tile shape).
    scratch: dict of 5 preallocated [128, 1024] tiles."""
    t = scratch["t"][:npart, :]
    kf = scratch["kf"][:npart, :]
    r = scratch["r"][:npart, :]
    rr = scratch["rr"][:npart, :]
    ki = scratch["ki"][:npart, :]
    v = nc.vector
    v.tensor_scalar(out=t[:], in0=in_, scalar1=INV_LN2, scalar2=MAGIC,
                    op0=OP.mult, op1=OP.add)
    v.tensor_scalar(out=kf[:], in0=t[:], scalar1=MAGIC, scalar2=None, op0=OP.subtract)
    v.scalar_tensor_tensor(out=r[:], in0=kf[:], scalar=-LN2_HI, in1=in_,
                           op0=OP.mult, op1=OP.add)
    v.scalar_tensor_tensor(out=rr[:], in0=kf[:], scalar=-LN2_LO, in1=r[:],
                           op0=OP.mult, op1=OP.add)
    h = r
    v.memset(h[:], 0.0)
    for c in (1.0 / 720, 1.0 / 120, 1.0 / 24, 1.0 / 6, 0.5, 1.0):
        v.scalar_tensor_tensor(out=h[:], in0=h[:], scalar=float(c), in1=rr[:],
                               op0=OP.add, op1=OP.mult)
    v.tensor_scalar(out=h[:], in0=h[:], scalar1=1.0, scalar2=None, op0=OP.add)
    v.tensor_copy(ki[:], kf[:])
    v.tensor_scalar(out=ki[:], in0=ki[:], scalar1=127, scalar2=None, op0=OP.add)
    v.tensor_scalar(out=ki[:], in0=ki[:], scalar1=23, scalar2=None,
                    op0=OP.logical_shift_left)
    v.tensor_mul(out, h[:], ki[:].bitcast(F32))


def _patch_excess_waits(data: bytes) -> bytes:
    """walrus allows only ONE sync-wait command per instruction; move excess
    waits onto injected same-engine NoOps placed just before the offender."""
    d = json.loads(data)
    counter = [0]

    def fix_block(b):
        newlist = []
        for ins in b.get("instructions", []):
            si = ins.get("sync_info")
            ow = (si or {}).get("on_wait") or []
            if len(ow) > 1 and ins.get("engine") not in (None, "Unassigned"):
                for w in ow[:-1]:
                    newlist.append({
                        "debug": ins.get("debug", 0), "engine": ins["engine"],
                        "ins": [], "outs": [], "name": f"I-wsh{counter[0]}",
                        "opcode": "NoOp", "text_hint": "waitshield",
                        "sync_info": {"on_wait": [w], "on_update": []},
                    })
                    counter[0] += 1
                si["on_wait"] = [ow[-1]]
            newlist.append(ins)
        b["instructions"] = newlist
        for sub in b.get("blocks", []):
            fix_block(sub)

    for f in d["functions"]:
        blocks = f["blocks"]
        if isinstance(blocks, dict):
            blocks = [blocks]
        for blk in blocks:
            fix_block(blk)
    return json.dumps(d).encode()


def build_nc():
    nc = bass.Bass("TRN2", target_bir_lowering=False, debug=False, num_devices=NCORES)

    # ---------------- I/O ----------------
    xs_d = nc.dram_tensor("xs", (B * NS, D), F32, kind="ExternalInput")
    wah_d = nc.dram_tensor("wah", (128, 4, H), BF16, kind="ExternalInput")   # [p, dc, h] hi
    wal_d = nc.dram_tensor("wal", (128, 4, H), BF16, kind="ExternalInput")   # [p, dc, h] lo
    ba_d = nc.dram_tensor("ba", (128, 2), F32, kind="ExternalInput")         # [p, hc]
    wbr_d = nc.dram_tensor("wbr", (128, 2, C), F32, kind="ExternalInput")    # [p, hc, c]
    bbr_d = nc.dram_tensor("bbr", (C, 1), F32, kind="ExternalInput")
    w1_d = nc.dram_tensor("w1", (128, 2 * 4, H), F32, kind="ExternalInput")  # [p, c*4+dc, h]
    b1_d = nc.dram_tensor("b1", (128, 4), F32, kind="ExternalInput")         # [p, c*2+hc]
    w2_d = nc.dram_tensor("w2", (128, 4, 2), F32, kind="ExternalInput")      # [p, c*2+hc, o]
    b2_d = nc.dram_tensor("b2", (2, C), F32, kind="ExternalInput")           # [o, c]
    ident_d = nc.dram_tensor("ident", (128, 128), F32, kind="ExternalInput")
    ones1_d = nc.dram_tensor("ones1", (1, 128), F32, kind="ExternalInput")
    onesc_d = nc.dram_tensor("onesc", (128, 1), F32, kind="ExternalInput")
    onesm_d = nc.dram_tensor("onesm", (128, 128), BF16, kind="ExternalInput")
    ltri_d = nc.dram_tensor("ltri", (16, 16), F32, kind="ExternalInput")     # [a,p]=1 iff a<p
    boff_d = nc.dram_tensor("boff", (16, 4), F32, kind="ExternalInput")      # b*2048
    cbase_d = nc.dram_tensor("cbase", (128, 1), F32, kind="ExternalInput")   # core_id*2048
    brow_d = nc.dram_tensor("brow", (128, 1), F32, kind="ExternalInput")     # b(r)*2048
    lab_d = nc.dram_tensor("lab", (64, 1), I32, kind="ExternalInput")        # label per inst col
    bco_d = nc.dram_tensor("bco", (8, 1), I32, kind="ExternalInput")         # bc*8
    pidx_d = nc.dram_tensor("pidx", (128, 1), F32, kind="ExternalInput")     # p*128
    bcof_d = nc.dram_tensor("bcof", (8, 1), F32, kind="ExternalInput")       # bc*8 f32
    qoff_d = nc.dram_tensor("qoff", (16, 1), I32, kind="ExternalInput")      # i*16+q

    comb_o = nc.dram_tensor("out_comb", (B, N), F32, kind="ExternalOutput")
    thr_o = nc.dram_tensor("out_thr", (1, 4), F32, kind="ExternalOutput")
    sel_os = [nc.dram_tensor(f"out_sel{b}", (NS, D), F32, kind="ExternalOutput")
              for b in range(B)]
    cl_o = nc.dram_tensor("out_cl", (1, 1), F32, kind="ExternalOutput")
    cnt_o = nc.dram_tensor("out_cnt", (1, 4), F32, kind="ExternalOutput")

    with TileContext(nc) as tc:
        with (
            tc.tile_pool(name="const", bufs=1) as cp,
            tc.tile_pool(name="work", bufs=1) as wp,
            tc.tile_pool(name="dram", bufs=1, space="DRAM") as dr,
            tc.tile_pool(name="psB", bufs=2, space="PSUM") as psB,
        ):
            # ------------- constant loads -------------
            wah = cp.tile([128, 4, H], BF16, name="wah")
            nc.sync.dma_start(wah[:], wah_d.ap())
            wal = cp.tile([128, 4, H], BF16, name="wal")
            nc.sync.dma_start(wal[:], wal_d.ap())
            ba = cp.tile([128, 2], F32, name="ba")
            nc.sync.dma_start(ba[:], ba_d.ap())
            wbr = cp.tile([128, 2, C], F32, name="wbr")
            nc.sync.dma_start(wbr[:], wbr_d.ap())
            bbr = cp.tile([C, 1], F32, name="bbr")
            nc.sync.dma_start(bbr[:], bbr_d.ap())
            w1 = cp.tile([128, 8, H], F32, name="w1")
            nc.sync.dma_start(w1[:], w1_d.ap())
            b1 = cp.tile([128, 4], F32, name="b1")
            nc.sync.dma_start(b1[:], b1_d.ap())
            w2 = cp.tile([128, 4, 2], F32, name="w2")
            nc.sync.dma_start(w2[:], w2_d.ap())
            b2 = cp.tile([2, C], F32, name="b2")
            nc.sync.dma_start(b2[:], b2_d.ap())
            ident = cp.tile([128, 128], F32, name="ident")
            nc.sync.dma_start(ident[:], ident_d.ap())
            ones1 = cp.tile([1, 128], F32, name="ones1")
            nc.sync.dma_start(ones1[:], ones1_d.ap())
            onesc = cp.tile([128, 1], F32, name="onesc")
            nc.sync.dma_start(onesc[:], onesc_d.ap())
            onesm = cp.tile([128, 128], BF16, name="onesm")
            nc.sync.dma_start(onesm[:], onesm_d.ap())
            ltri = cp.tile([16, 16], F32, name="ltri")
            nc.sync.dma_start(ltri[:], ltri_d.ap())
            boff = cp.tile([16, 4], F32, name="boff")
            nc.sync.dma_start(boff[:], boff_d.ap())
            cbase = cp.tile([128, 1], F32, name="cbase")
            nc.sync.dma_start(cbase[:], cbase_d.ap())
            brow = cp.tile([128, 1], F32, name="brow")
            nc.sync.dma_start(brow[:], brow_d.ap())
            lab = cp.tile([64, 1], I32, name="lab")
            nc.sync.dma_start(lab[:], lab_d.ap())
            bco = cp.tile([8, 1], I32, name="bco")
            nc.sync.dma_start(bco[:], bco_d.ap())
            pidx = cp.tile([128, 1], F32, name="pidx")
            nc.sync.dma_start(pidx[:], pidx_d.ap())
            bcof = cp.tile([8, 1], F32, name="bcof")
            nc.sync.dma_start(bcof[:], bcof_d.ap())
            qoff = cp.tile([16, 1], I32, name="qoff")
            nc.sync.dma_start(qoff[:], qoff_d.ap())

            # Lsb and Lg (later) share one 64KB/partition slot via tag
            Lsb = wp.tile([C, B, NS], F32, name="Lsb", tag="bigslot")  # [c, b, n]
            # allocate the indirect-DMA bounds register before collectives
            # grab gpsimd's register file
            bc_reg = nc.gpsimd.to_reg(B * NS - 1)
            bc_reg_ns = nc.gpsimd.to_reg(NS - 1)
            exp_scr = {
                "t": wp.tile([128, 1024], F32, name="exp_t"),
                "kf": wp.tile([128, 1024], F32, name="exp_kf"),
                "r": wp.tile([128, 1024], F32, name="exp_r"),
                "rr": wp.tile([128, 1024], F32, name="exp_rr"),
                "ki": wp.tile([128, 1024], I32, name="exp_ki"),
            }

            # dram scratch
            cc_in = dr.tile([C, B, NS], F32, name="cc_in")
            cc_out = dr.tile([NCORES, C, B, NS], F32, name="cc_out")
            ci_in = dr.tile([128, D], F32, name="ci_in")
            ci_out = dr.tile([128, D], F32, name="ci_out")
            cvd = dr.tile([128, 64], F32, name="cvd")
            cvnd = dr.tile([128, 64], F32, name="cvnd")
            qd2 = dr.tile([128, 1], I32, name="qd2")      # flat positions bounce
            cboth = dr.tile([256, 64], I32, name="cboth") # candN (top) ++ candNn (bot)
            maskd = dr.tile([128, 4, 128], F32, name="maskd")  # global mask bounce

            # ============ Phase A: GEMM over 16 chunks of 512 rows ============
            # X -> bf16 hi/lo split on DVE, DMA-transpose (xbar) to get
            # contraction dim on partitions, 3-pass bf16 matmul (hi*hi +
            # hi*lo + lo*hi) accumulated in fp32 PSUM.
            xs_flat = xs_d.ap()  # (8192, 512)
            with (
                tc.tile_pool(name="xp", bufs=2) as xp,
                tc.tile_pool(name="xtp", bufs=2) as xtp,
                tc.tile_pool(name="htp", bufs=3) as htp,
                tc.tile_pool(name="psA", bufs=1, space="PSUM") as psA,
            ):
                for ch in range(16):
                    b_i, u = ch // 4, ch % 4
                    Xc = xp.tile([128, 4, D], F32, name="Xc", tag="Xc", bufs=2)
                    src = xs_flat[ch * 512:(ch + 1) * 512, :].rearrange(
                        "(s p) d -> p s d", p=128)
                    nc.sync.dma_start(Xc[:], src)
                    Xhi = xp.tile([128, 4, D], BF16, name="Xhi", tag="Xhi", bufs=3)
                    nc.vector.tensor_copy(Xhi[:], Xc[:])
                    Xlo = xp.tile([128, 4, D], BF16, name="Xlo", tag="Xlo", bufs=3)
                    nc.vector.tensor_tensor(out=Xlo[:], in0=Xc[:], in1=Xhi[:],
                                            op=OP.subtract)
                    # one xbar transpose per operand: out[p, e, c] = in[c, e*128+p]
                    # with in free f = s*512+d  ->  e = s*4+dc, p = d%128
                    XTh = xtp.tile([128, 16, 128], BF16, name="XTh", tag="XTh", bufs=4)
                    XTl = xtp.tile([128, 16, 128], BF16, name="XTl", tag="XTl", bufs=4)
                    nc.sync.dma_start_transpose(
                        XTh[:], Xhi[:].rearrange("p s d -> p (s d)"))
                    nc.sync.dma_start_transpose(
                        XTl[:], Xlo[:].rearrange("p s d -> p (s d)"))
                    aps = psA.tile([C, 512], F32, name="aps", tag="aps", bufs=2)
                    for hc in range(2):
                        hps = psA.tile([128, 512], F32, name="hps", tag="hps", bufs=2)
                        first = True
                        for dc in range(4):
                            for lhs, rhs in ((wah, XTh), (wah, XTl), (wal, XTh)):
                                nc.tensor.matmul(
                                    out=hps[:], lhsT=lhs[:, dc, hc * 128:(hc + 1) * 128],
                                    rhs=rhs[:, dc::4, :],
                                    start=first, stop=(dc == 3 and lhs is wal))
                                first = False
                        hT = htp.tile([128, 512], F32, name="hT", tag="hT", bufs=3)
                        nc.scalar.activation(hT[:], hps[:], ACTF.Relu, bias=ba[:, hc:hc + 1])
                        nc.tensor.matmul(out=aps[:], lhsT=wbr[:, hc, :], rhs=hT[:],
                                         start=(hc == 0), stop=(hc == 1))
                    nc.vector.tensor_scalar(out=Lsb[:, b_i, u * 512:(u + 1) * 512],
                                            in0=aps[:], scalar1=bbr[:, :1], scalar2=None,
                                            op0=OP.add)

            # ============ Phase B: AllGather logits ============
            nc.sync.dma_start(cc_in[:], Lsb[:])
            nc.gpsimd.collective_compute(
                "AllGather", OP.bypass, replica_groups=[list(range(NCORES))],
                ins=[cc_in.opt()], outs=[cc_out.opt()],
            )

            # ============ Phase C: global softmax / combined ============
            Lt = wp.tile([128, 8, 128], F32, name="Lt")
            for r in range(NCORES):
                nc.sync.dma_start(
                    Lt[r * 16:(r + 1) * 16, :, :],
                    cc_out[r].rearrange("c b (q j) -> q (c b) j", j=128))
            Et = wp.tile([128, 8, 128], F32, name="Et")
            _emit_exp(nc, exp_scr, Et[:].rearrange("p a b -> p (a b)"),
                      Lt[:].rearrange("p a b -> p (a b)"), 128)
            partial = wp.tile([128, 8], F32, name="partial")
            nc.vector.tensor_reduce(out=partial[:], in_=Et[:], axis=AX.X, op=OP.add)
            tpp = psB.tile([8, 128], F32, name="tpp", tag="ps")
            nc.tensor.transpose(tpp[:], partial[:], ident[:])
            pt_sb = wp.tile([8, 128], F32, name="pt_sb")
            nc.vector.tensor_copy(pt_sb[:], tpp[:])
            S = wp.tile([8, 1], F32, name="S")
            nc.vector.tensor_reduce(out=S[:], in_=pt_sb[:], axis=AX.X, op=OP.add)
            invS = wp.tile([8, 1], F32, name="invS")
            nc.vector.reciprocal(invS[:], S[:])
            tiv = psB.tile([1, 8], F32, name="tiv", tag="ps")
            nc.tensor.transpose(tiv[:], invS[:], ident[:8, :8])
            iv_sb = wp.tile([1, 8], F32, name="iv_sb")
            nc.vector.tensor_copy(iv_sb[:], tiv[:])
            pib = psB.tile([128, 8], F32, name="pib", tag="ps")
            nc.tensor.matmul(out=pib[:], lhsT=ones1[:], rhs=iv_sb[:], start=True, stop=True)
            invB = wp.tile([128, 8], F32, name="invB")
            nc.vector.tensor_copy(invB[:], pib[:])
            # P = E * invS, in place over Et
            nc.vector.tensor_mul(Et[:], Et[:], invB[:, :, None].to_broadcast([128, 8, 128]))
            comb = wp.tile([128, 4, 128], F32, name="comb")
            nc.vector.tensor_add(comb[:], Et[:, 0:4, :], Et[:, 4:8, :])
            nc.vector.tensor_scalar(out=comb[:], in0=comb[:], scalar1=0.5, scalar2=None,
                                    op0=OP.mult)

            # ============ Phase D: binary search for per-b thresholds ============
            m1 = wp.tile([128, 4], F32, name="m1")
            nc.vector.tensor_reduce(out=m1[:], in_=comb[:], axis=AX.X, op=OP.max)
            tm1 = psB.tile([4, 128], F32, name="tm1", tag="ps")
            nc.tensor.transpose(tm1[:], m1[:], ident[:])
            tm1_sb = wp.tile([4, 128], F32, name="tm1_sb")
            nc.vector.tensor_copy(tm1_sb[:], tm1[:])
            m2 = wp.tile([4, 1], F32, name="m2")
            nc.vector.tensor_reduce(out=m2[:], in_=tm1_sb[:], axis=AX.X, op=OP.max)
            tm2 = psB.tile([1, 4], F32, name="tm2", tag="ps")
            nc.tensor.transpose(tm2[:], m2[:], ident[:4, :4])
            hi14 = wp.tile([1, 4], F32, name="hi14")
            nc.vector.tensor_copy(hi14[:], tm2[:])
            pib2 = psB.tile([128, 4], F32, name="pib2", tag="ps")
            nc.tensor.matmul(out=pib2[:], lhsT=ones1[:], rhs=hi14[:], start=True, stop=True)
            hi = wp.tile([128, 4], F32, name="hi")
            nc.vector.tensor_copy(hi[:], pib2[:])
            lo = wp.tile([128, 4], F32, name="lo")
            nc.vector.memset(lo[:], 0.0)
            mid = wp.tile([128, 4], F32, name="mid")
            gts = wp.tile([128, 4, 128], F32, name="gts")
            cnt128 = wp.tile([128, 4], BF16, name="cnt128")
            ge = wp.tile([128, 4], U32, name="ge")
            lt = wp.tile([128, 4], U32, name="lt")
            for it in range(SEARCH_ITERS):
                nc.vector.tensor_add(mid[:], lo[:], hi[:])
                nc.vector.tensor_scalar(out=mid[:], in0=mid[:], scalar1=0.5, scalar2=None,
                                        op0=OP.mult)
                for b_i in range(4):
                    nc.vector.tensor_scalar(out=gts[:, b_i, :], in0=comb[:, b_i, :],
                                            scalar1=mid[:, b_i:b_i + 1], scalar2=None,
                                            op0=OP.is_gt)
                with nc.allow_low_precision("counts <= 128 are exact in bf16"):
                    nc.vector.tensor_reduce(out=cnt128[:], in_=gts[:], axis=AX.X, op=OP.add)
                # replicated total: every partition gets sum over partitions
                pcn = psB.tile([128, 4], F32, name="pcn", tag="ps")
                nc.tensor.matmul(out=pcn[:], lhsT=onesm[:], rhs=cnt128[:], start=True, stop=True)
                nc.vector.tensor_scalar(out=ge[:], in0=pcn[:], scalar1=float(K),
                                        scalar2=None, op0=OP.is_ge)
                nc.vector.tensor_scalar(out=lt[:], in0=pcn[:], scalar1=float(K),
                                        scalar2=None, op0=OP.is_lt)
                nc.vector.copy_predicated(lo[:], ge[:], mid[:])
                nc.vector.copy_predicated(hi[:], lt[:], mid[:])
            nc.sync.dma_start(thr_o.ap(), lo[0:1, :])
            # final count (debug/assert)
            thr128 = lo
            for b_i in range(4):
                nc.vector.tensor_scalar(out=gts[:, b_i, :], in0=comb[:, b_i, :],
                                        scalar1=thr128[:, b_i:b_i + 1], scalar2=None,
                                        op0=OP.is_gt)
            with nc.allow_low_precision("counts <= 128 are exact in bf16"):
                nc.vector.tensor_reduce(out=cnt128[:], in_=gts[:], axis=AX.X, op=OP.add)
            pcf = psB.tile([128, 4], F32, name="pcf", tag="ps")
            nc.tensor.matmul(out=pcf[:], lhsT=onesm[:], rhs=cnt128[:], start=True, stop=True)
            cntf = wp.tile([1, 4], F32, name="cntf")
            nc.vector.tensor_copy(cntf[:], pcf[0:1, :])
            nc.sync.dma_start(cnt_o.ap(), cntf[:])
            nc.sync.dma_start(maskd[:], gts[:])
            # write global combined
            nc.sync.dma_start(
                comb_o.ap().rearrange("b (p j) -> p b j", p=128), comb[:])

            # ============ Phase E: local compaction ============
            # gather this core's 16 partition-rows of the global mask from the
            # dram bounce (row offsets are a per-core input -> static code,
            # and the mask is by construction identical to the host's
            # comb > thr comparison)
            maskL = wp.tile([16, 4, 128], F32, name="maskL")
            nc.gpsimd.indirect_dma_start(
                out=maskL[:].rearrange("q a b -> q (a b)"),
                out_offset=None,
                in_=maskd[:].rearrange("p a b -> p (a b)"),
                in_offset=bass.IndirectOffsetOnAxis(ap=qoff[:, :1], axis=0),
                bounds_check=None, oob_is_err=True)
            zeros16 = wp.tile([16, 128], F32, name="zeros16")
            nc.vector.memset(zeros16[:], 0.0)
            pref = wp.tile([16, 4, 128], F32, name="pref")
            for b_i in range(4):
                nc.vector.tensor_tensor_scan(out=pref[:, b_i, :], data0=maskL[:, b_i, :],
                                             data1=zeros16[:], initial=0.0,
                                             op0=OP.add, op1=OP.add)
            Ops = psB.tile([16, 4], F32, name="Ops", tag="ps")
            nc.tensor.matmul(out=Ops[:], lhsT=ltri[:], rhs=pref[:, :, 127], start=True, stop=True)
            obb = wp.tile([16, 4], F32, name="obb")
            nc.vector.tensor_add(obb[:], Ops[:], boff[:])
            pos = wp.tile([16, 4, 128], F32, name="pos")
            nc.vector.tensor_sub(pos[:], pref[:], maskL[:])
            nc.vector.tensor_add(pos[:], pos[:], obb[:, :, None].to_broadcast([16, 4, 128]))
            bigsel = wp.tile([16, 4, 128], F32, name="bigsel")
            nc.vector.memset(bigsel[:], BIG)
            maskLi = wp.tile([16, 4, 128], U32, name="maskLi")
            nc.vector.tensor_copy(maskLi[:], maskL[:])
            posx = wp.tile([16, 4, 128], F32, name="posx")
            nc.vector.select(posx[:], maskLi[:], pos[:], bigsel[:])
            post = wp.tile([128, 4, 16], I32, name="post")
            for b_i in range(4):
                ptp = psB.tile([128, 16], F32, name="ptp", tag="ps")
                nc.tensor.transpose(ptp[:], posx[:, b_i, :], ident[:16, :16])
                nc.vector.tensor_copy(post[:, b_i, :], ptp[:])

            # ============ Phase F: masked scatter of selected rows ============
            with tc.tile_pool(name="fp", bufs=12) as fp:
                for t in range(64):
                    b_i, q = t % 4, t // 4   # interleave b so the 4 WAW chains pipeline
                    tt = b_i * 16 + q
                    Xs = fp.tile([128, D], F32, name="Xs", tag="Xs", bufs=12)
                    nc.sync.dma_start(Xs[:], xs_flat[tt * 128:(tt + 1) * 128, :])
                    nc.gpsimd.indirect_dma_start(
                        out=sel_os[b_i].ap(),
                        out_offset=bass.IndirectOffsetOnAxis(
                            ap=post[:, b_i, q:q + 1], axis=0),
                        in_=Xs[:],
                        in_offset=None,
                        bounds_check=bc_reg_ns,
                        oob_is_err=False,
                    )

            # ============ Phase G: instance top8/bot8 + gather ============
            # hierarchical: per-partition top8 candidates on the (128, 8, 128)
            # layout, global top8 over the 1024 candidates, index recovery via
            # a small DRAM bounce + indirect gather of the candidate ids.
            candV = wp.tile([128, 8, 8], F32, name="candV")
            candJ = wp.tile([128, 8, 8], U32, name="candJ")
            candN = wp.tile([128, 8, 8], I32, name="candN")
            candVn = wp.tile([128, 8, 8], F32, name="candVn")
            candJn = wp.tile([128, 8, 8], U32, name="candJn")
            candNn = wp.tile([128, 8, 8], I32, name="candNn")
            for bc in range(8):
                nc.vector.max(out=candV[:, bc, :], in_=Lt[:, bc, :])
                nc.vector.max_index(out=candJ[:, bc, :], in_max=candV[:, bc, :],
                                    in_values=Lt[:, bc, :])
            candNf = wp.tile([128, 8, 8], F32, name="candNf")
            nc.vector.tensor_scalar(out=candNf[:].rearrange("p a b -> p (a b)"),
                                    in0=candJ[:].rearrange("p a b -> p (a b)"),
                                    scalar1=pidx[:, :1], scalar2=None, op0=OP.add)
            nc.vector.tensor_copy(candN[:].rearrange("p a b -> p (a b)"),
                                  candNf[:].rearrange("p a b -> p (a b)"))
            nc.vector.tensor_scalar(out=Lt[:].rearrange("p a b -> p (a b)"),
                                    in0=Lt[:].rearrange("p a b -> p (a b)"),
                                    scalar1=-1.0, scalar2=None, op0=OP.mult)
            for bc in range(8):
                nc.vector.max(out=candVn[:, bc, :], in_=Lt[:, bc, :])
                nc.vector.max_index(out=candJn[:, bc, :],
                                    in_max=candVn[:, bc, :], in_values=Lt[:, bc, :])
            nc.vector.tensor_scalar(out=candNf[:].rearrange("p a b -> p (a b)"),
                                    in0=candJn[:].rearrange("p a b -> p (a b)"),
                                    scalar1=pidx[:, :1], scalar2=None, op0=OP.add)
            nc.vector.tensor_copy(candNn[:].rearrange("p a b -> p (a b)"),
                                  candNf[:].rearrange("p a b -> p (a b)"))
            nc.sync.dma_start(cvd[:], candV[:].rearrange("p a b -> p (a b)"))
            nc.sync.dma_start(cboth[0:128, :], candN[:].rearrange("p a b -> p (a b)"))
            nc.sync.dma_start(cvnd[:], candVn[:].rearrange("p a b -> p (a b)"))
            nc.sync.dma_start(cboth[128:256, :], candNn[:].rearrange("p a b -> p (a b)"))
            candVg = wp.tile([8, 128, 8], F32, name="candVg")
            candVgn = wp.tile([8, 128, 8], F32, name="candVgn")
            nc.sync.dma_start(candVg[:], cvd[:].rearrange("p (a r) -> a p r", a=8))
            nc.sync.dma_start(candVgn[:], cvnd[:].rearrange("p (a r) -> a p r", a=8))
            gtop = wp.tile([8, 8], F32, name="gtop")
            gq = wp.tile([8, 8], U32, name="gq")
            nc.vector.max(out=gtop[:], in_=candVg[:])
            nc.vector.max_index(out=gq[:], in_max=gtop[:],
                                in_values=candVg[:].rearrange("a p r -> a (p r)"))
            gtopn = wp.tile([8, 8], F32, name="gtopn")
            gqn = wp.tile([8, 8], U32, name="gqn")
            nc.vector.max(out=gtopn[:], in_=candVgn[:])
            nc.vector.max_index(out=gqn[:], in_max=gtopn[:],
                                in_values=candVgn[:].rearrange("a p r -> a (p r)"))
            # flat position in the cnd dram bounce: (q>>3)*64 + bc*8 + (q&7)
            flats = wp.tile([8, 2, 8], I32, name="flats")
            for src_q, tb in ((gq, 0), (gqn, 1)):
                f1 = wp.tile([8, 8], U32, name=f"f1_{tb}")
                nc.vector.tensor_scalar(out=f1[:], in0=src_q[:], scalar1=3,
                                        scalar2=6, op0=OP.logical_shift_right,
                                        op1=OP.logical_shift_left)
                f2 = wp.tile([8, 8], U32, name=f"f2_{tb}")
                nc.vector.tensor_scalar(out=f2[:], in0=src_q[:], scalar1=7,
                                        scalar2=None, op0=OP.bitwise_and)
                nc.vector.tensor_add(f1[:], f1[:], f2[:])
                ff = wp.tile([8, 8], F32, name=f"ff_{tb}")
                nc.vector.tensor_scalar(out=ff[:], in0=f1[:],
                                        scalar1=bcof[:, :1],
                                        scalar2=(128.0 * 64 if tb == 1 else 0.0),
                                        op0=OP.add, op1=OP.add)
                nc.vector.tensor_copy(flats[:, tb, :], ff[:])
            nc.sync.dma_start(qd2[:].rearrange("(a k) o -> a (k o)", a=8),
                              flats[:].rearrange("a b c -> a (b c)"))
            flat128 = wp.tile([128, 1], I32, name="flat128")
            nc.sync.dma_start(flat128[:], qd2[:])
            ng128 = wp.tile([128, 1], I32, name="ng128")
            nc.gpsimd.indirect_dma_start(
                out=ng128[:], out_offset=None,
                in_=cboth[:].rearrange("p k -> (p k)")[:, None],
                in_offset=bass.IndirectOffsetOnAxis(ap=flat128[:, :1], axis=0),
                bounds_check=None, oob_is_err=True)
            ngf = wp.tile([128, 1], F32, name="ngf")
            nc.vector.tensor_copy(ngf[:], ng128[:])
            t0 = wp.tile([128, 1], F32, name="t0")
            nc.vector.tensor_scalar(out=t0[:], in0=ngf[:], scalar1=cbase[:, :1],
                                    scalar2=None, op0=OP.subtract)
            v1 = wp.tile([128, 1], U32, name="v1")
            nc.vector.tensor_scalar(out=v1[:], in0=t0[:], scalar1=0.0, scalar2=None,
                                    op0=OP.is_ge)
            v2 = wp.tile([128, 1], U32, name="v2")
            nc.vector.tensor_scalar(out=v2[:], in0=t0[:], scalar1=float(NS), scalar2=None,
                                    op0=OP.is_lt)
            nc.vector.tensor_mul(v1[:], v1[:], v2[:])
            ladd = wp.tile([128, 1], F32, name="ladd")
            nc.vector.tensor_add(ladd[:], t0[:], brow[:])
            big1 = wp.tile([128, 1], F32, name="big1")
            nc.vector.memset(big1[:], BIG)
            lidxf = wp.tile([128, 1], F32, name="lidxf")
            nc.vector.select(lidxf[:], v1[:], ladd[:], big1[:])
            lidx = wp.tile([128, 1], I32, name="lidx")
            nc.vector.tensor_copy(lidx[:], lidxf[:])
            inst = wp.tile([128, D], F32, name="inst")
            nc.vector.memset(inst[:], 0.0)
            nc.gpsimd.indirect_dma_start(
                out=inst[:], out_offset=None,
                in_=xs_d.ap(),
                in_offset=bass.IndirectOffsetOnAxis(ap=lidx[:, :1], axis=0),
                bounds_check=bc_reg, oob_is_err=False,
            )
            nc.sync.dma_start(ci_in[:], inst[:])
            nc.gpsimd.collective_compute(
                "AllReduce", OP.add, replica_groups=[list(range(NCORES))],
                ins=[ci_in.opt()], outs=[ci_out.opt()],
            )
            inst2 = wp.tile([128, D], F32, name="inst2")
            nc.sync.dma_start(inst2[:], ci_out[:])

            # ============ Phase H: instance MLP + CE ============
            instT = wp.tile([128, 4, 128], F32, name="instT")
            for dc in range(4):
                itp = psB.tile([128, 128], F32, name="itp", tag="ps")
                nc.tensor.transpose(itp[:], inst2[:, dc * 128:(dc + 1) * 128], ident[:])
                nc.scalar.copy(instT[:, dc, :], itp[:])
            clp = psB.tile([1, 1], F32, name="clp", tag="clp", bufs=1)
            for c_i in range(2):
                h1s = []
                for hc in range(2):
                    hp1 = psB.tile([128, 64], F32, name="hp1", tag="ps")
                    for dc in range(4):
                        nc.tensor.matmul(out=hp1[:], lhsT=w1[:, c_i * 4 + dc, hc * 128:(hc + 1) * 128],
                                         rhs=instT[:, dc, c_i * 64:(c_i + 1) * 64],
                                         start=(dc == 0), stop=(dc == 3))
                    h1sb = wp.tile([128, 64], F32, name=f"h1sb{c_i}{hc}")
                    nc.scalar.activation(h1sb[:], hp1[:], ACTF.Relu,
                                         bias=b1[:, c_i * 2 + hc:c_i * 2 + hc + 1])
                    h1s.append(h1sb)
                l2p = psB.tile([2, 64], F32, name="l2p", tag="ps")
                for hc in range(2):
                    nc.tensor.matmul(out=l2p[:], lhsT=w2[:, c_i * 2 + hc, :], rhs=h1s[hc][:],
                                     start=(hc == 0), stop=(hc == 1))
                l2s = wp.tile([2, 64], F32, name=f"l2s{c_i}")
                nc.vector.tensor_scalar(out=l2s[:], in0=l2p[:], scalar1=b2[:, c_i:c_i + 1],
                                        scalar2=None, op0=OP.add)
                l2tp = psB.tile([64, 2], F32, name="l2tp", tag="ps")
                nc.tensor.transpose(l2tp[:], l2s[:], ident[:2, :2])
                l2T = wp.tile([64, 2], F32, name=f"l2T{c_i}")
                nc.vector.tensor_copy(l2T[:], l2tp[:])
                mx = wp.tile([64, 1], F32, name=f"mx{c_i}")
                nc.vector.tensor_reduce(out=mx[:], in_=l2T[:], axis=AX.X, op=OP.max)
                mneg = wp.tile([64, 1], F32, name=f"mneg{c_i}")
                nc.vector.tensor_scalar(out=mneg[:], in0=mx[:], scalar1=-1.0, scalar2=None,
                                        op0=OP.mult)
                ex = wp.tile([64, 2], F32, name=f"ex{c_i}")
                nc.scalar.activation(ex[:], l2T[:], ACTF.Exp, bias=mneg[:, :1])
                sm = wp.tile([64, 1], F32, name=f"sm{c_i}")
                nc.vector.tensor_reduce(out=sm[:], in_=ex[:], axis=AX.X, op=OP.add)
                lse = wp.tile([64, 1], F32, name=f"lse{c_i}")
                nc.scalar.activation(lse[:], sm[:], ACTF.Ln)
                lsel = wp.tile([64, 1], F32, name=f"lsel{c_i}")
                nc.vector.select(lsel[:], lab[:], l2T[:, 1:2], l2T[:, 0:1])
                ce = wp.tile([64, 1], F32, name=f"ce{c_i}")
                nc.vector.tensor_add(ce[:], mx[:], lse[:])
                nc.vector.tensor_sub(ce[:], ce[:], lsel[:])
                nc.tensor.matmul(out=clp[:], lhsT=ce[:], rhs=onesc[:64, :],
                                 start=(c_i == 0), stop=(c_i == 1))
            cls = wp.tile([1, 1], F32, name="cls")
            nc.scalar.activation(cls[:], clp[:], ACTF.Copy, scale=1.0 / 128)
            nc.sync.dma_start(cl_o.ap(), cls[:])

    return nc


_NC_CACHE = None


def _get_nc():
    global _NC_CACHE
    if _NC_CACHE is None:
        nc = build_nc()
        orig = nc.to_json_bytes
        nc.to_json_bytes = lambda: _patch_excess_waits(orig())
        _NC_CACHE = nc
    return _NC_CACHE


def _host_inputs(features, Wa, ba, Wbr, bbr, W1, b1, W2, b2):
    """Build the per-core input maps."""
    import ml_dtypes
    _wa = Wa.reshape(4, 128, H).transpose(1, 0, 2).copy()
    _wa_hi = _wa.astype(ml_dtypes.bfloat16)
    _wa_lo = (_wa - _wa_hi.astype(np.float32)).astype(ml_dtypes.bfloat16)
    common = {
        "wah": _wa_hi,
        "wal": _wa_lo,
        "ba": ba.reshape(2, 128).T.copy(),
        "wbr": Wbr.T.reshape(2, 128, C).transpose(1, 0, 2).copy(),
        "bbr": bbr[:, None].copy(),
        "w1": W1.reshape(C * 4, 128, H).transpose(1, 0, 2).copy(),
        "b1": b1.reshape(C, 2, 128).transpose(2, 0, 1).reshape(128, 4).copy(),
        "w2": W2.reshape(C * 2, 128, 2).transpose(1, 0, 2).copy(),
        "b2": b2.T.copy(),
        "ident": np.eye(128, dtype=np.float32),
        "ones1": np.ones((1, 128), dtype=np.float32),
        "onesc": np.ones((128, 1), dtype=np.float32),
        "onesm": np.ones((128, 128), dtype=ml_dtypes.bfloat16),
        "ltri": np.triu(np.ones((16, 16), dtype=np.float32), k=1),
        "boff": np.zeros((16, 4), dtype=np.float32),
        "brow": (np.repeat(np.arange(8) % 4, 16).astype(np.float32) * NS)[:, None],
        "lab": np.tile(np.r_[np.ones(8), np.zeros(8)].astype(np.int32), 4)[:, None],
        "bco": (np.arange(8, dtype=np.int32) * 8)[:, None],
        "pidx": (np.arange(128, dtype=np.float32) * 128)[:, None],
        "bcof": (np.arange(8, dtype=np.float32) * 8)[:, None],
    }
    in_maps = []
    for i in range(NCORES):
        m = dict(common)
        m["xs"] = np.ascontiguousarray(
            features[:, i * NS:(i + 1) * NS, :].reshape(B * NS, D))
        m["cbase"] = np.full((128, 1), i * NS, dtype=np.float32)
        m["qoff"] = (np.arange(16, dtype=np.int32) + i * 16)[:, None]
        in_maps.append(m)
    return in_maps


def kernel(features, Wa, ba, Wbr, bbr, W1, b1, W2, b2):
    features = np.asarray(features, dtype=np.float32)
    in_maps = _host_inputs(features, np.asarray(Wa), np.asarray(ba), np.asarray(Wbr),
                           np.asarray(bbr), np.asarray(W1), np.asarray(b1),
                           np.asarray(W2), np.asarray(b2))
    nc = _get_nc()
    trace = bool(int(os.environ.get("KERNEL_TRACE", "0")))
    if trace:
        try:
            from antenv.axon_hooks import set_axon_ntff_profile_hook
            from trn_agent_boot.trn_boot import _ntff_profile_via_ctypes
            set_axon_ntff_profile_hook(_ntff_profile_via_ctypes("/opt/axon/libaxon_pjrt.so"))
        except Exception as e:
            print("ntff hook setup failed:", e)
    res = run_bass_kernel_spmd(nc, in_maps, core_ids=list(range(NCORES)), trace=trace)
    if trace:
        kernel.last_result = res
    r0 = res.results[0]
    comb = r0["out_comb"]                      # (4, 16384)
    thr = r0["out_thr"][0]                     # (4,)
    mask = comb > thr[:, None]
    counts = mask.sum(axis=1)
    assert np.all(counts == K), f"threshold search failed: counts={counts}"
    sel_idx = np.zeros((B, K), dtype=np.int32)
    selected = np.empty((B, K, D), dtype=np.float32)
    for b_i in range(B):
        idx = np.nonzero(mask[b_i])[0]
        sel_idx[b_i] = idx.astype(np.int32)
        off = 0
        for i in range(NCORES):
            c = int(mask[b_i, i * NS:(i + 1) * NS].sum())
            rows = res.results[i][f"out_sel{b_i}"][:c, :]
            selected[b_i, off:off + c] = rows
            off += c
        assert off == K
    cl_loss = np.float32(r0["out_cl"][0, 0])
    return selected, comb, sel_idx, cl_loss
